# revision 3
# baseline (speedup 1.0000x reference)
"""CNN-LSTM (VGG16 features + LSTM + linear head), data-parallel over batch on 8 NeuronCores.

Strategy:
- fp16 hi/lo split arithmetic everywhere on the TensorEngine (fp32-grade accuracy at
  1 cycle/row): conv1_1 via host im2col with K=108 (27 taps*channels x 4 hi/lo terms),
  conv1_2/conv2_1 via stacked-K [Xhi|Xlo] (2 matmuls/tap, full product), deeper layers
  via 3-variant (hh, hl, lh) matmuls.
- Activations live in DRAM in zero-padded [C, T, H+2, W+2] layouts (hi/lo fp16);
  maxpools fused into the preceding conv's epilogue; exact interior windows via
  strided APs (no pad compute).
- LSTM: input projection as 36 matmuls -> [T, 4H] psum; recurrence with h kept via
  DVE 32x32 transpose; gates on free dim of one partition; head fused at the end.
"""
import os
import numpy as np

T = 32
HD = 32
AFT = None  # set on import of mybir inside kernel

_last_results = None

# layer configs: (name, cin, cout, H, W, pool, mode)
#   H, W = conv spatial (input == output); pool: output halved into next buffer
CFGS = [
    ("c11", 3, 64, 96, 32, False, "im2col"),
    ("c12", 64, 64, 96, 32, True, "stk1"),     # per-frame path
    ("c21", 64, 128, 48, 16, False, "stk"),
    ("c22", 128, 128, 48, 16, True, "3var"),
    ("c31", 128, 256, 24, 8, False, "3var"),
    ("c32", 256, 256, 24, 8, False, "3var"),
    ("c33", 256, 256, 24, 8, True, "3var"),
    ("c41", 256, 512, 12, 4, False, "3var"),
    ("c42", 512, 512, 12, 4, False, "3var"),
    ("c43", 512, 512, 12, 4, True, "3var"),
    ("c51", 512, 512, 6, 2, False, "3var"),
    ("c52", 512, 512, 6, 2, False, "3var"),
    ("c53", 512, 512, 6, 2, True, "3var"),
]

# chunking for the whole-stage (B) path: stage spatial -> (fb, rows) chunk shape
# s2: per-frame row-split; s3..s5: frame-blocks, full frames
CHUNKS = {
    (48, 16): [("rows", 0, 24), ("rows", 24, 24)],   # per frame: (r0, rows)
    (24, 8): [("frames", f0, 2) for f0 in range(0, 32, 2)],
    (12, 4): [("frames", f0, 8) for f0 in range(0, 32, 8)],
    (6, 2): [("frames", 0, 32)],
}


def _split16(x):
    hi = x.astype(np.float16)
    lo = (x.astype(np.float32) - hi.astype(np.float32)).astype(np.float16)
    return hi, lo


def _prep_host(images, vgg_params, lstm_params, head_params):
    """Host-side data marshalling: im2col for conv1_1, weight layouts, biases."""
    B = images.shape[0]
    ins_common = {}
    # --- vgg weights ---
    wi = 0
    for (name, cin, cout, H, W, pool, mode) in CFGS:
        Wt, b = vgg_params[wi]
        Wt = np.asarray(Wt, np.float32)
        b = np.asarray(b, np.float32)
        wi += 1
        # lhsT layout [ci, tap, co]
        wT = Wt.transpose(1, 2, 3, 0).reshape(cin, 9, cout)
        whi, wlo = _split16(wT)
        if mode == "im2col":
            # single K=108 lhsT: rows [Whi; Whi; Wlo; Wlo] with ci-major rows k=t*3+c
            w27 = Wt.transpose(2, 3, 1, 0).reshape(9 * cin, cout)  # [t*3+c? no:]
            # careful: transpose(2,3,1,0) gives [dy, dx, ci, co] -> reshape [9*ci, co] with k=(dy*3+dx)*3+ci
            whi27, wlo27 = _split16(w27)
            ins_common["w_c11"] = np.concatenate([whi27, whi27, wlo27, wlo27], 0)  # [108, 64]
        elif mode in ("stk1", "stk"):
            ins_common[f"w_{name}_1"] = np.concatenate([whi, whi], 0)  # [128, 9, cout]
            ins_common[f"w_{name}_2"] = np.concatenate([wlo, wlo], 0)
        else:
            G = cin // 128
            ins_common[f"w_{name}_hi"] = whi.reshape(G, 128, 9, cout)
            ins_common[f"w_{name}_lo"] = wlo.reshape(G, 128, 9, cout)
        Go = max(1, cout // 128)
        cpad = b.reshape(Go, -1).T.copy() if cout >= 128 else b.reshape(1, cout).T.copy()
        ins_common[f"b_{name}"] = np.ascontiguousarray(cpad, np.float32)  # [<=128, Go]

    # --- lstm ---
    Wih, Whh, bih, bhh = [np.asarray(a, np.float32) for a in lstm_params]
    wih_dev = np.zeros((3, 4, 2, 128, 128), np.float16)
    Wih_cy = Wih.reshape(4 * HD, 512, 3)
    for y in range(3):
        for g in range(4):
            blk = Wih_cy[:, g * 128:(g + 1) * 128, y].T
            bh, bl_ = _split16(blk)
            wih_dev[y, g, 0] = bh
            wih_dev[y, g, 1] = bl_
    ins_common["wih"] = wih_dev
    ins_common["whh"] = Whh.T.astype(np.float16)                # [32, 128]
    ins_common["bsum"] = (bih + bhh).astype(np.float32).reshape(1, 4 * HD)
    Wl, bl = [np.asarray(a, np.float32) for a in head_params[0]]
    wl_dev = np.zeros((HD, 2, 2), np.float16)
    WlT = Wl.T
    wl_dev[:, 0, :], wl_dev[:, 1, :] = _split16(WlT)
    ins_common["wl"] = wl_dev
    ins_common["bl"] = bl.reshape(1, 2).astype(np.float32)

    # --- per-core im2col [108, T, 98, 34] fp16 ---
    per_core = []
    imgs = np.asarray(images, np.float32)
    for bidx in range(B):
        x = imgs[bidx]                                  # [T, 3, 96, 32]
        xpad = np.zeros((3, T, 98, 34), np.float32)
        xpad[:, :, 1:97, 1:33] = x.transpose(1, 0, 2, 3)
        flat = xpad.reshape(3, T, 98 * 34)
        hi, lo = _split16(flat)
        i2c = np.zeros((108, T, 98 * 34), np.float16)
        for dy in range(3):
            for dx in range(3):
                off = dy * 34 + dx
                tksl = slice(0, 3332 - off)
                for c in range(3):
                    k = (dy * 3 + dx) * 3 + c
                    i2c[k, :, tksl] = hi[c, :, off:]
                    i2c[27 + k, :, tksl] = lo[c, :, off:]
                    i2c[54 + k, :, tksl] = hi[c, :, off:]
                    i2c[81 + k, :, tksl] = lo[c, :, off:]
        per_core.append({"i2c": i2c.reshape(108, T, 98, 34)})
    return ins_common, per_core


def _build_module():
    import concourse.bass as bass
    import concourse.bacc as bacc
    import concourse.mybir as mybir
    import concourse.tile as tile
    AF = mybir.ActivationFunctionType
    AO = mybir.AluOpType
    f16, f32 = mybir.dt.float16, mybir.dt.float32

    nc = bacc.Bacc("TRN2", target_bir_lowering=False, debug=False, num_devices=8)

    # ---- dram tensors ----
    i2c = nc.dram_tensor("i2c", [108, T, 98, 34], f16, kind="ExternalInput")
    wdr = {}
    wdr["w_c11"] = nc.dram_tensor("w_c11", [108, 64], f16, kind="ExternalInput")
    bias_dr = {}
    for (name, cin, cout, H, W, pool, mode) in CFGS:
        Go = max(1, cout // 128)
        bias_dr[name] = nc.dram_tensor(f"b_{name}", [min(cout, 128), Go], f32, kind="ExternalInput")
        if mode in ("stk1", "stk"):
            wdr[f"w_{name}_1"] = nc.dram_tensor(f"w_{name}_1", [128, 9, cout], f16, kind="ExternalInput")
            wdr[f"w_{name}_2"] = nc.dram_tensor(f"w_{name}_2", [128, 9, cout], f16, kind="ExternalInput")
        elif mode == "3var":
            G = cin // 128
            wdr[f"w_{name}_hi"] = nc.dram_tensor(f"w_{name}_hi", [G, 128, 9, cout], f16, kind="ExternalInput")
            wdr[f"w_{name}_lo"] = nc.dram_tensor(f"w_{name}_lo", [G, 128, 9, cout], f16, kind="ExternalInput")
    wih = nc.dram_tensor("wih", [3, 4, 2, 128, 128], f16, kind="ExternalInput")
    whh = nc.dram_tensor("whh", [HD, 4 * HD], f16, kind="ExternalInput")
    bsum = nc.dram_tensor("bsum", [1, 4 * HD], f32, kind="ExternalInput")
    wl = nc.dram_tensor("wl", [HD, 2, 2], f16, kind="ExternalInput")
    bl = nc.dram_tensor("bl", [1, 2], f32, kind="ExternalInput")
    out = nc.dram_tensor("out", [T, 2], f32, kind="ExternalOutput")

    # activation buffers (internal DRAM), zero-filled at start
    def dbuf(nm, shape):
        return nc.dram_tensor(nm, shape, f16, kind="Internal")
    a1 = dbuf("a1", [128, T, 98, 34])            # c11 out, stacked hi|lo (64+64)
    s2in = dbuf("s2in", [128, T, 50, 18])        # pool1 out, stacked
    s2b = [dbuf("s2b_hi", [1, 128, T, 50, 18]), dbuf("s2b_lo", [1, 128, T, 50, 18])]
    s3in = [dbuf("s3in_hi", [1, 128, T, 26, 10]), dbuf("s3in_lo", [1, 128, T, 26, 10])]
    s3b = [dbuf("s3b_hi", [2, 128, T, 26, 10]), dbuf("s3b_lo", [2, 128, T, 26, 10])]
    s3c = [dbuf("s3c_hi", [2, 128, T, 26, 10]), dbuf("s3c_lo", [2, 128, T, 26, 10])]
    s4in = [dbuf("s4in_hi", [2, 128, T, 14, 6]), dbuf("s4in_lo", [2, 128, T, 14, 6])]
    s4b = [dbuf("s4b_hi", [4, 128, T, 14, 6]), dbuf("s4b_lo", [4, 128, T, 14, 6])]
    s4c = [dbuf("s4c_hi", [4, 128, T, 14, 6]), dbuf("s4c_lo", [4, 128, T, 14, 6])]
    s5in = [dbuf("s5in_hi", [4, 128, T, 8, 4]), dbuf("s5in_lo", [4, 128, T, 8, 4])]
    s5b = [dbuf("s5b_hi", [4, 128, T, 8, 4]), dbuf("s5b_lo", [4, 128, T, 8, 4])]
    s5c = [dbuf("s5c_hi", [4, 128, T, 8, 4]), dbuf("s5c_lo", [4, 128, T, 8, 4])]

    with tile.TileContext(nc) as tc, \
            tc.tile_pool(name="persist", bufs=1) as persist, \
            tc.tile_pool(name="ps", bufs=1, space="PSUM") as pspool:
        # ---- zero-fill pad buffers ----
        zt = persist.tile([128, 8192], f16)
        nc.vector.memset(zt[:], 0.0)
        for buf in ([a1, s2in] + s2b + s3in + s3b + s3c + s4in + s4b + s4c + s5in + s5b + s5c):
            sh = buf.shape
            if len(sh) == 4:
                per = sh[1] * sh[2] * sh[3]
                flat = buf.ap().rearrange("p a b c -> p (a b c)")
                for o in range(0, per, 8192):
                    n = min(8192, per - o)
                    nc.gpsimd.dma_start(flat[:, o:o + n], zt[:, :n])
            else:
                per = sh[2] * sh[3] * sh[4]
                for g in range(sh[0]):
                    flat = buf.ap()[g].rearrange("p a b c -> p (a b c)")
                    for o in range(0, per, 8192):
                        n = min(8192, per - o)
                        nc.gpsimd.dma_start(flat[:, o:o + n], zt[:, :n])

        # bias tiles
        bias_t = {}
        for (name, cin, cout, H, W, pool, mode) in CFGS:
            Go = max(1, cout // 128)
            bt = persist.tile([min(cout, 128), Go], f32, name=f"bt_{name}")
            nc.sync.dma_start(bt[:], bias_dr[name].ap())
            bias_t[name] = bt

        feats = persist.tile([128, 2, 4, T, 3], f16)

        # ================= stage 1 (per-frame path) =================
        with tc.tile_pool(name="s1w", bufs=1) as s1w:
            w11t = s1w.tile([108, 64], f16)
            nc.sync.dma_start(w11t[:], wdr["w_c11"].ap())
            w12t1 = s1w.tile([128, 9, 64], f16)
            w12t2 = s1w.tile([128, 9, 64], f16)
            nc.sync.dma_start(w12t1[:], wdr["w_c12_1"].ap())
            nc.sync.dma_start(w12t2[:], wdr["w_c12_2"].ap())

            with tc.tile_pool(name="s1a", bufs=1) as s1a:
                for f in range(T):
                    # ---- conv1_1 ----
                    imt = s1a.tile([108, 98, 34], f16, tag="imt", bufs=2, name=f"imt{f}")
                    nc.sync.dma_start(imt[:], i2c.ap()[:, f])
                    for ci, r0 in enumerate(range(0, 96, 16)):
                        p = pspool.tile([64, 512], f32, tag="mm", bufs=4, name=f"p11_{f}_{ci}")
                        nc.tensor.matmul(p[:].rearrange("c (a b) -> c a b", a=16),
                                         w11t[:], imt[:, r0:r0 + 16, 0:32],
                                         start=True, stop=True)
                        y = s1a.tile([64, 16, 32], f32, tag="y1", bufs=3, name=f"y11_{f}_{ci}")
                        nc.scalar.activation(y[:].rearrange("c a b -> c (a b)"), p[:],
                                             AF.Relu, bias=bias_t["c11"][:64, 0:1])
                        hi = s1a.tile([64, 16, 32], f16, tag="hi1", bufs=3, name=f"h11_{f}_{ci}")
                        lo = s1a.tile([64, 16, 32], f16, tag="lo1", bufs=3, name=f"l11_{f}_{ci}")
                        nc.vector.tensor_copy(hi[:], y[:])
                        nc.vector.tensor_sub(lo[:], y[:], hi[:])
                        nc.sync.dma_start(a1.ap()[0:64, f, r0 + 1:r0 + 17, 1:33], hi[:])
                        nc.sync.dma_start(a1.ap()[64:128, f, r0 + 1:r0 + 17, 1:33], lo[:])
                    # ---- conv1_2 + pool1 ----
                    at = s1a.tile([128, 98, 34], f16, tag="at", bufs=2, name=f"at{f}")
                    nc.sync.dma_start(at[:], a1.ap()[:, f])
                    for ci, r0 in enumerate(range(0, 96, 16)):
                        p = pspool.tile([64, 512], f32, tag="mm", bufs=4, name=f"p12_{f}_{ci}")
                        mm = 0
                        for dy in range(3):
                            for dx in range(3):
                                t_ = dy * 3 + dx
                                rhs = at[:, r0 + dy:r0 + dy + 16, dx:dx + 32]
                                po = p[:].rearrange("c (a b) -> c a b", a=16)
                                nc.tensor.matmul(po, w12t1[:, t_, :], rhs,
                                                 start=(mm == 0), stop=False)
                                mm += 1
                                nc.tensor.matmul(po, w12t2[:, t_, :], rhs,
                                                 start=False, stop=(mm == 17))
                                mm += 1
                        y = s1a.tile([64, 16, 32], f32, tag="y1", bufs=3, name=f"y12_{f}_{ci}")
                        nc.scalar.activation(y[:].rearrange("c a b -> c (a b)"), p[:],
                                             AF.Relu, bias=bias_t["c12"][:64, 0:1])
                        yv = y[:].rearrange("c (h p) (w q) -> c h p w q", p=2, q=2)
                        m1 = s1a.tile([64, 8, 16], f32, tag="m1", bufs=3, name=f"m1_{f}_{ci}")
                        m2 = s1a.tile([64, 8, 16], f32, tag="m2", bufs=3, name=f"m2_{f}_{ci}")
                        nc.vector.tensor_tensor(m1[:], yv[:, :, 0, :, 0], yv[:, :, 0, :, 1], AO.max)
                        nc.vector.tensor_tensor(m2[:], yv[:, :, 1, :, 0], yv[:, :, 1, :, 1], AO.max)
                        nc.vector.tensor_tensor(m1[:], m1[:], m2[:], AO.max)
                        hi = s1a.tile([64, 8, 16], f16, tag="hi1p", bufs=3, name=f"h12_{f}_{ci}")
                        lo = s1a.tile([64, 8, 16], f16, tag="lo1p", bufs=3, name=f"l12_{f}_{ci}")
                        nc.vector.tensor_copy(hi[:], m1[:])
                        nc.vector.tensor_sub(lo[:], m1[:], hi[:])
                        rp = r0 // 2
                        nc.sync.dma_start(s2in.ap()[0:64, f, rp + 1:rp + 9, 1:17], hi[:])
                        nc.sync.dma_start(s2in.ap()[64:128, f, rp + 1:rp + 9, 1:17], lo[:])

        # ================= stages 2-5 (whole-stage path) =================
        def conv_layer(name, cin, cout, H, W, pool, mode, src, dst):
            """src: stacked tensor (mode=='stk') or [hi, lo] grp'd tensors; dst likewise."""
            Hp, Wp = H + 2, W + 2
            Gi = max(1, cin // 128)
            Go = max(1, cout // 128)
            with tc.tile_pool(name=f"L_{name}", bufs=1) as lp:
                # load acts
                if mode == "stk":
                    at_hi = [lp.tile([128, T, Hp, Wp], f16, name=f"a_{name}")]
                    nc.sync.dma_start(at_hi[0][:], src.ap())
                    at_lo = None
                else:
                    at_hi, at_lo = [], []
                    for g in range(Gi):
                        th = lp.tile([128, T, Hp, Wp], f16, name=f"ah_{name}_{g}")
                        tl = lp.tile([128, T, Hp, Wp], f16, name=f"al_{name}_{g}")
                        nc.sync.dma_start(th[:], src[0].ap()[g])
                        nc.sync.dma_start(tl[:], src[1].ap()[g])
                        at_hi.append(th)
                        at_lo.append(tl)
                # load weights
                if mode == "stk":
                    w1 = lp.tile([128, 9, cout], f16, name=f"w1_{name}")
                    w2 = lp.tile([128, 9, cout], f16, name=f"w2_{name}")
                    nc.sync.dma_start(w1[:], wdr[f"w_{name}_1"].ap())
                    nc.sync.dma_start(w2[:], wdr[f"w_{name}_2"].ap())
                else:
                    whi_t = lp.tile([128, Gi, 9, cout], f16, name=f"wh_{name}")
                    wlo_t = lp.tile([128, Gi, 9, cout], f16, name=f"wl_{name}")
                    nc.sync.dma_start(whi_t[:], wdr[f"w_{name}_hi"].ap().rearrange("g p t c -> p g t c"))
                    nc.sync.dma_start(wlo_t[:], wdr[f"w_{name}_lo"].ap().rearrange("g p t c -> p g t c"))

                chunks = []
                if (H, W) == (48, 16):
                    for f in range(T):
                        for (kind, r0, rows) in CHUNKS[(48, 16)]:
                            chunks.append((f, 1, r0, rows))
                else:
                    for (kind, f0, fb) in CHUNKS[(H, W)]:
                        chunks.append((f0, fb, 0, H))

                for go in range(Go):
                    cosl = slice(go * 128, go * 128 + min(128, cout))
                    Mp = min(128, cout)
                    for (f0, fb, r0, rows) in chunks:
                        N = fb * rows * W
                        p = pspool.tile([128, 512], f32, tag="mm", bufs=4,
                                        name=f"p_{name}_{go}_{f0}_{r0}")
                        pv = p[:Mp, :N].rearrange("c (f a b) -> c f a b", f=fb, a=rows)
                        mm = 0
                        nmm = 9 * Gi * (2 if mode == "stk" else 3)
                        for gi in range(Gi):
                            for dy in range(3):
                                for dx in range(3):
                                    t_ = dy * 3 + dx
                                    if mode == "stk":
                                        rhs = at_hi[0][:, f0:f0 + fb, r0 + dy:r0 + dy + rows, dx:dx + W]
                                        nc.tensor.matmul(pv, w1[:, t_, cosl], rhs,
                                                         start=(mm == 0), stop=(mm == nmm - 1))
                                        mm += 1
                                        nc.tensor.matmul(pv, w2[:, t_, cosl], rhs,
                                                         start=False, stop=(mm == nmm - 1))
                                        mm += 1
                                    else:
                                        rh = at_hi[gi][:, f0:f0 + fb, r0 + dy:r0 + dy + rows, dx:dx + W]
                                        rl = at_lo[gi][:, f0:f0 + fb, r0 + dy:r0 + dy + rows, dx:dx + W]
                                        nc.tensor.matmul(pv, whi_t[:, gi, t_, cosl], rh,
                                                         start=(mm == 0), stop=False)
                                        mm += 1
                                        nc.tensor.matmul(pv, wlo_t[:, gi, t_, cosl], rh,
                                                         start=False, stop=False)
                                        mm += 1
                                        nc.tensor.matmul(pv, whi_t[:, gi, t_, cosl], rl,
                                                         start=False, stop=(mm == nmm - 1))
                                        mm += 1
                        y = lp.tile([128, fb, rows, W], f32, tag="y", bufs=3,
                                    name=f"y_{name}_{go}_{f0}_{r0}")
                        nc.scalar.activation(y[:Mp].rearrange("c f a b -> c (f a b)"), p[:Mp, :N],
                                             AF.Relu, bias=bias_t[name][:Mp, go:go + 1])
                        if not pool:
                            hi = lp.tile([128, fb, rows, W], f16, tag="sh", bufs=3, name=f"sh_{name}_{go}_{f0}_{r0}")
                            lo = lp.tile([128, fb, rows, W], f16, tag="sl", bufs=3, name=f"sl_{name}_{go}_{f0}_{r0}")
                            nc.vector.tensor_copy(hi[:Mp], y[:Mp])
                            nc.vector.tensor_sub(lo[:Mp], y[:Mp], hi[:Mp])
                            for fi in range(fb):
                                nc.sync.dma_start(dst[0].ap()[go, 0:Mp, f0 + fi, r0 + 1:r0 + rows + 1, 1:W + 1], hi[:Mp, fi])
                                nc.sync.dma_start(dst[1].ap()[go, 0:Mp, f0 + fi, r0 + 1:r0 + rows + 1, 1:W + 1], lo[:Mp, fi])
                        else:
                            yv = y[:Mp].rearrange("c f (h p) (w q) -> c f h p w q", p=2, q=2)
                            ph, pw = rows // 2, W // 2
                            m1 = lp.tile([128, fb, ph, pw], f32, tag="pm1", bufs=3, name=f"pm1_{name}_{go}_{f0}_{r0}")
                            m2 = lp.tile([128, fb, ph, pw], f32, tag="pm2", bufs=3, name=f"pm2_{name}_{go}_{f0}_{r0}")
                            nc.vector.tensor_tensor(m1[:Mp], yv[:, :, :, 0, :, 0], yv[:, :, :, 0, :, 1], AO.max)
                            nc.vector.tensor_tensor(m2[:Mp], yv[:, :, :, 1, :, 0], yv[:, :, :, 1, :, 1], AO.max)
                            nc.vector.tensor_tensor(m1[:Mp], m1[:Mp], m2[:Mp], AO.max)
                            hi = lp.tile([128, fb, ph, pw], f16, tag="sh", bufs=3, name=f"sh_{name}_{go}_{f0}_{r0}")
                            lo = lp.tile([128, fb, ph, pw], f16, tag="sl", bufs=3, name=f"sl_{name}_{go}_{f0}_{r0}")
                            nc.vector.tensor_copy(hi[:Mp], m1[:Mp])
                            nc.vector.tensor_sub(lo[:Mp], m1[:Mp], hi[:Mp])
                            if name == "c53":
                                nc.vector.tensor_copy(feats[:, 0, go], hi[:Mp].rearrange("c f a b -> c f (a b)"))
                                nc.vector.tensor_copy(feats[:, 1, go], lo[:Mp].rearrange("c f a b -> c f (a b)"))
                            else:
                                rp = r0 // 2
                                for fi in range(fb):
                                    nc.sync.dma_start(
                                        dst[0].ap()[go, 0:Mp, f0 + fi, rp + 1:rp + ph + 1, 1:pw + 1], hi[:Mp, fi])
                                    nc.sync.dma_start(
                                        dst[1].ap()[go, 0:Mp, f0 + fi, rp + 1:rp + ph + 1, 1:pw + 1], lo[:Mp, fi])

        conv_layer("c21", 64, 128, 48, 16, False, "stk", s2in, s2b)
        conv_layer("c22", 128, 128, 48, 16, True, "3var", s2b, s3in)
        conv_layer("c31", 128, 256, 24, 8, False, "3var", s3in, s3b)
        conv_layer("c32", 256, 256, 24, 8, False, "3var", s3b, s3c)
        conv_layer("c33", 256, 256, 24, 8, True, "3var", s3c, s4in)
        conv_layer("c41", 256, 512, 12, 4, False, "3var", s4in, s4b)
        conv_layer("c42", 512, 512, 12, 4, False, "3var", s4b, s4c)
        conv_layer("c43", 512, 512, 12, 4, True, "3var", s4c, s5in)
        conv_layer("c51", 512, 512, 6, 2, False, "3var", s5in, s5b)
        conv_layer("c52", 512, 512, 6, 2, False, "3var", s5b, s5c)
        conv_layer("c53", 512, 512, 6, 2, True, "3var", s5c, None)

        # ================= LSTM + head =================
        import concourse.bass as bass
        with tc.tile_pool(name="lstm", bufs=1) as sb:
            wt = sb.tile([128, 3, 4, 2, 128], f16)
            nc.sync.dma_start(wt[:], wih.ap().rearrange("y g v c k -> c y g v k"))
            whht = sb.tile([HD, 4 * HD], f16)
            nc.sync.dma_start(whht[:], whh.ap())
            wlt = sb.tile([HD, 2, 2], f16)
            nc.sync.dma_start(wlt[:], wl.ap())

            pg = pspool.tile([T, 4 * HD], f32, tag="pg", name="pg")
            first = True
            for y in range(3):
                for g in range(4):
                    for (fv, wv) in ((0, 0), (0, 1), (1, 0)):
                        nc.tensor.matmul(pg[:], feats[:, fv, g, :, y], wt[:, y, g, wv, :],
                                         start=first, stop=(y == 2 and g == 3 and (fv, wv) == (1, 0)))
                        first = False
            binp32 = sb.tile([T, 4 * HD], f32)
            bsum_bc_t = sb.tile([T, 4 * HD], f32)
            nc.gpsimd.dma_start(out=bsum_bc_t[:], in_=bass.AP(
                tensor=bsum.ap().tensor, offset=0, ap=[[0, T], [1, 4 * HD]]))
            nc.vector.tensor_tensor(binp32[:], pg[:], bsum_bc_t[:], mybir.AluOpType.add)
            binp_flat = sb.tile([1, T * 4 * HD], f32)
            nc.sync.dma_start(binp_flat[:], binp32[:])

            h_sq = sb.tile([HD, HD], f16)
            h_sqT = sb.tile([HD, HD], f16)
            nc.vector.memset(h_sq[:], 0.0)
            nc.vector.memset(h_sqT[:], 0.0)
            cst = sb.tile([1, HD], f32)
            nc.vector.memset(cst[:], 0.0)
            gact = sb.tile([1, 4 * HD], f32)
            tmp1 = sb.tile([1, HD], f32)
            tmp2 = sb.tile([1, HD], f32)
            tanhc = sb.tile([1, HD], f32)
            h32 = sb.tile([1, HD], f32)
            HT = sb.tile([HD, T], f16)
            gsum = sb.tile([1, 4 * HD], f32)

            for t in range(T):
                pr = pspool.tile([1, 4 * HD], f32, tag="pr", bufs=2, name=f"pr{t}")
                nc.tensor.matmul(pr[:], h_sqT[:, 0:1], whht[:], start=True, stop=True)
                nc.vector.tensor_tensor(gsum[:], pr[:], binp_flat[:, t * 4 * HD:(t + 1) * 4 * HD],
                                        mybir.AluOpType.add)
                nc.scalar.activation(gact[:, 0:2 * HD], gsum[:, 0:2 * HD], AF.Sigmoid)
                nc.scalar.activation(gact[:, 2 * HD:3 * HD], gsum[:, 2 * HD:3 * HD], AF.Tanh)
                nc.scalar.activation(gact[:, 3 * HD:4 * HD], gsum[:, 3 * HD:4 * HD], AF.Sigmoid)
                nc.vector.tensor_tensor(tmp1[:], gact[:, HD:2 * HD], cst[:], mybir.AluOpType.mult)
                nc.vector.tensor_tensor(tmp2[:], gact[:, 0:HD], gact[:, 2 * HD:3 * HD], mybir.AluOpType.mult)
                nc.vector.tensor_tensor(cst[:], tmp1[:], tmp2[:], mybir.AluOpType.add)
                nc.scalar.activation(tanhc[:], cst[:], AF.Tanh)
                nc.vector.tensor_tensor(h32[:], gact[:, 3 * HD:4 * HD], tanhc[:], mybir.AluOpType.mult)
                nc.vector.tensor_copy(h_sq[0:1, :], h32[:])
                nc.vector.transpose(h_sqT[:], h_sq[:])
                nc.vector.tensor_copy(HT[:, t:t + 1], h_sqT[:, 0:1])

            HrT = sb.tile([HD, T], f16)
            nc.scalar.activation(HrT[:], HT[:], AF.Relu)
            Hr32 = sb.tile([HD, T], f32)
            nc.scalar.activation(Hr32[:], HT[:], AF.Relu)
            HrLo = sb.tile([HD, T], f16)
            nc.vector.tensor_sub(HrLo[:], Hr32[:], HrT[:])
            po = pspool.tile([T, 2], f32, tag="po", name="po")
            nc.tensor.matmul(po[:], HrT[:], wlt[:, 0, :], start=True, stop=False)
            nc.tensor.matmul(po[:], HrT[:], wlt[:, 1, :], start=False, stop=False)
            nc.tensor.matmul(po[:], HrLo[:], wlt[:, 0, :], start=False, stop=True)
            ot = sb.tile([T, 2], f32)
            bl_bc_t = sb.tile([T, 2], f32)
            nc.gpsimd.dma_start(out=bl_bc_t[:], in_=bass.AP(
                tensor=bl.ap().tensor, offset=0, ap=[[0, T], [1, 2]]))
            nc.vector.tensor_tensor(ot[:], po[:], bl_bc_t[:], mybir.AluOpType.add)
            nc.sync.dma_start(out.ap(), ot[:])

    nc.compile()
    return nc


def kernel(images, vgg_params, lstm_params, head_params):
    global _last_results
    import sys
    sys.path.insert(0, os.path.dirname(os.path.abspath(__file__)))
    try:
        import axon_prof
        axon_prof.install()
    except Exception:
        pass
    from concourse.bass_utils import run_bass_kernel_spmd

    images = np.asarray(images)
    B = images.shape[0]
    ins_common, per_core = _prep_host(images, vgg_params, lstm_params, head_params)
    nc = _build_module()
    in_maps = [{**ins_common, **per_core[b]} for b in range(B)]
    res = run_bass_kernel_spmd(nc, in_maps, core_ids=list(range(8)))
    _last_results = res
    standing = np.stack([res.results[b]["out"] for b in range(B)], axis=0).astype(np.float32)
    return (standing, standing, standing, standing)


# revision 4
# speedup vs baseline: 6.2385x; 6.2385x over previous
"""CNN-LSTM (VGG16 features + LSTM + linear head), data-parallel over batch on 8 NeuronCores.

Strategy:
- fp16 hi/lo split arithmetic everywhere on the TensorEngine (fp32-grade accuracy at
  1 cycle/row): conv1_1 via host im2col with K=108 (27 taps*channels x 4 hi/lo terms),
  conv1_2/conv2_1 via stacked-K [Xhi|Xlo] (2 matmuls/tap, full product), deeper layers
  via 3-variant (hh, hl, lh) matmuls.
- Activations live in DRAM in zero-padded [C, T, H+2, W+2] layouts (hi/lo fp16);
  maxpools fused into the preceding conv's epilogue; exact interior windows via
  strided APs (no pad compute).
- LSTM: input projection as 36 matmuls -> [T, 4H] psum; recurrence with h kept via
  DVE 32x32 transpose; gates on free dim of one partition; head fused at the end.
"""
import os
import numpy as np

T = 32
HD = 32
AFT = None  # set on import of mybir inside kernel

_last_results = None

# layer configs: (name, cin, cout, H, W, pool, mode)
#   H, W = conv spatial (input == output); pool: output halved into next buffer
CFGS = [
    ("c11", 3, 64, 96, 32, False, "im2col"),
    ("c12", 64, 64, 96, 32, True, "stk1"),     # per-frame path
    ("c21", 64, 128, 48, 16, False, "stk"),
    ("c22", 128, 128, 48, 16, True, "3var"),
    ("c31", 128, 256, 24, 8, False, "3var"),
    ("c32", 256, 256, 24, 8, False, "3var"),
    ("c33", 256, 256, 24, 8, True, "3var"),
    ("c41", 256, 512, 12, 4, False, "3var"),
    ("c42", 512, 512, 12, 4, False, "3var"),
    ("c43", 512, 512, 12, 4, True, "3var"),
    ("c51", 512, 512, 6, 2, False, "3var"),
    ("c52", 512, 512, 6, 2, False, "3var"),
    ("c53", 512, 512, 6, 2, True, "3var"),
]

# chunking for the whole-stage (B) path: stage spatial -> (fb, rows) chunk shape
# s2: per-frame row-split; s3..s5: frame-blocks, full frames
CHUNKS = {
    (48, 16): [("rows", 0, 24), ("rows", 24, 24)],   # per frame: (r0, rows)
    (24, 8): [("frames", f0, 2) for f0 in range(0, 32, 2)],
    (12, 4): [("frames", f0, 8) for f0 in range(0, 32, 8)],
    (6, 2): [("frames", 0, 32)],
}


def _split16(x):
    hi = x.astype(np.float16)
    lo = (x.astype(np.float32) - hi.astype(np.float32)).astype(np.float16)
    return hi, lo


def _prep_host(images, vgg_params, lstm_params, head_params):
    """Host-side data marshalling: im2col for conv1_1, weight layouts, biases."""
    B = images.shape[0]
    ins_common = {}
    # --- vgg weights ---
    wi = 0
    for (name, cin, cout, H, W, pool, mode) in CFGS:
        Wt, b = vgg_params[wi]
        Wt = np.asarray(Wt, np.float32)
        b = np.asarray(b, np.float32)
        wi += 1
        # lhsT layout [ci, tap, co]
        wT = Wt.transpose(1, 2, 3, 0).reshape(cin, 9, cout)
        whi, wlo = _split16(wT)
        if mode == "im2col":
            # single K=108 lhsT: rows [Whi; Whi; Wlo; Wlo] with ci-major rows k=t*3+c
            w27 = Wt.transpose(2, 3, 1, 0).reshape(9 * cin, cout)  # [t*3+c? no:]
            # careful: transpose(2,3,1,0) gives [dy, dx, ci, co] -> reshape [9*ci, co] with k=(dy*3+dx)*3+ci
            whi27, wlo27 = _split16(w27)
            ins_common["w_c11"] = np.concatenate([whi27, whi27, wlo27, wlo27], 0)  # [108, 64]
        elif mode in ("stk1", "stk"):
            ins_common[f"w_{name}_1"] = np.concatenate([whi, whi], 0)  # [128, 9, cout]
            ins_common[f"w_{name}_2"] = np.concatenate([wlo, wlo], 0)
        else:
            G = cin // 128
            ins_common[f"w_{name}_hi"] = whi.reshape(G, 128, 9, cout)
            ins_common[f"w_{name}_lo"] = wlo.reshape(G, 128, 9, cout)
        Go = max(1, cout // 128)
        cpad = b.reshape(Go, -1).T.copy() if cout >= 128 else b.reshape(1, cout).T.copy()
        ins_common[f"b_{name}"] = np.ascontiguousarray(cpad, np.float32)  # [<=128, Go]

    # --- lstm ---
    Wih, Whh, bih, bhh = [np.asarray(a, np.float32) for a in lstm_params]
    wih_dev = np.zeros((3, 4, 2, 128, 128), np.float16)
    Wih_cy = Wih.reshape(4 * HD, 512, 3)
    for y in range(3):
        for g in range(4):
            blk = Wih_cy[:, g * 128:(g + 1) * 128, y].T
            bh, bl_ = _split16(blk)
            wih_dev[y, g, 0] = bh
            wih_dev[y, g, 1] = bl_
    ins_common["wih"] = wih_dev
    ins_common["whh"] = Whh.T.astype(np.float16)                # [32, 128]
    ins_common["bsum"] = (bih + bhh).astype(np.float32).reshape(1, 4 * HD)
    Wl, bl = [np.asarray(a, np.float32) for a in head_params[0]]
    wl_dev = np.zeros((HD, 2, 2), np.float16)
    WlT = Wl.T
    wl_dev[:, 0, :], wl_dev[:, 1, :] = _split16(WlT)
    ins_common["wl"] = wl_dev
    ins_common["bl"] = bl.reshape(1, 2).astype(np.float32)

    # --- per-core im2col [108, T, 98, 34] fp16 ---
    per_core = []
    imgs = np.asarray(images, np.float32)
    for bidx in range(B):
        x = imgs[bidx]                                  # [T, 3, 96, 32]
        xpad = np.zeros((3, T, 98, 34), np.float32)
        xpad[:, :, 1:97, 1:33] = x.transpose(1, 0, 2, 3)
        flat = xpad.reshape(3, T, 98 * 34)
        hi, lo = _split16(flat)
        i2c = np.zeros((108, T, 98 * 34), np.float16)
        for dy in range(3):
            for dx in range(3):
                off = dy * 34 + dx
                tksl = slice(0, 3332 - off)
                for c in range(3):
                    k = (dy * 3 + dx) * 3 + c
                    i2c[k, :, tksl] = hi[c, :, off:]
                    i2c[27 + k, :, tksl] = lo[c, :, off:]
                    i2c[54 + k, :, tksl] = hi[c, :, off:]
                    i2c[81 + k, :, tksl] = lo[c, :, off:]
        per_core.append({"i2c": i2c.reshape(108, T, 98, 34)})
    return ins_common, per_core


def _build_module():
    import concourse.bass as bass
    import concourse.bacc as bacc
    import concourse.mybir as mybir
    import concourse.tile as tile
    AF = mybir.ActivationFunctionType
    AO = mybir.AluOpType
    f16, f32 = mybir.dt.float16, mybir.dt.float32

    nc = bacc.Bacc("TRN2", target_bir_lowering=False, debug=False, num_devices=8)

    # ---- dram tensors ----
    i2c = nc.dram_tensor("i2c", [108, T, 98, 34], f16, kind="ExternalInput")
    wdr = {}
    wdr["w_c11"] = nc.dram_tensor("w_c11", [108, 64], f16, kind="ExternalInput")
    bias_dr = {}
    for (name, cin, cout, H, W, pool, mode) in CFGS:
        Go = max(1, cout // 128)
        bias_dr[name] = nc.dram_tensor(f"b_{name}", [min(cout, 128), Go], f32, kind="ExternalInput")
        if mode in ("stk1", "stk"):
            wdr[f"w_{name}_1"] = nc.dram_tensor(f"w_{name}_1", [128, 9, cout], f16, kind="ExternalInput")
            wdr[f"w_{name}_2"] = nc.dram_tensor(f"w_{name}_2", [128, 9, cout], f16, kind="ExternalInput")
        elif mode == "3var":
            G = cin // 128
            wdr[f"w_{name}_hi"] = nc.dram_tensor(f"w_{name}_hi", [G, 128, 9, cout], f16, kind="ExternalInput")
            wdr[f"w_{name}_lo"] = nc.dram_tensor(f"w_{name}_lo", [G, 128, 9, cout], f16, kind="ExternalInput")
    wih = nc.dram_tensor("wih", [3, 4, 2, 128, 128], f16, kind="ExternalInput")
    whh = nc.dram_tensor("whh", [HD, 4 * HD], f16, kind="ExternalInput")
    bsum = nc.dram_tensor("bsum", [1, 4 * HD], f32, kind="ExternalInput")
    wl = nc.dram_tensor("wl", [HD, 2, 2], f16, kind="ExternalInput")
    bl = nc.dram_tensor("bl", [1, 2], f32, kind="ExternalInput")
    out = nc.dram_tensor("out", [T, 2], f32, kind="ExternalOutput")

    # activation buffers (internal DRAM), zero-filled at start
    def dbuf(nm, shape):
        return nc.dram_tensor(nm, shape, f16, kind="Internal")
    a1 = dbuf("a1", [128, T, 98, 34])            # c11 out, stacked hi|lo (64+64)
    s2in = dbuf("s2in", [128, T, 50, 18])        # pool1 out, stacked
    s2b = [dbuf("s2b_hi", [1, 128, T, 50, 18]), dbuf("s2b_lo", [1, 128, T, 50, 18])]
    s3in = [dbuf("s3in_hi", [1, 128, T, 26, 10]), dbuf("s3in_lo", [1, 128, T, 26, 10])]
    s3b = [dbuf("s3b_hi", [2, 128, T, 26, 10]), dbuf("s3b_lo", [2, 128, T, 26, 10])]
    s3c = [dbuf("s3c_hi", [2, 128, T, 26, 10]), dbuf("s3c_lo", [2, 128, T, 26, 10])]
    s4in = [dbuf("s4in_hi", [2, 128, T, 14, 6]), dbuf("s4in_lo", [2, 128, T, 14, 6])]
    s4b = [dbuf("s4b_hi", [4, 128, T, 14, 6]), dbuf("s4b_lo", [4, 128, T, 14, 6])]
    s4c = [dbuf("s4c_hi", [4, 128, T, 14, 6]), dbuf("s4c_lo", [4, 128, T, 14, 6])]
    s5in = [dbuf("s5in_hi", [4, 128, T, 8, 4]), dbuf("s5in_lo", [4, 128, T, 8, 4])]
    s5b = [dbuf("s5b_hi", [4, 128, T, 8, 4]), dbuf("s5b_lo", [4, 128, T, 8, 4])]
    s5c = [dbuf("s5c_hi", [4, 128, T, 8, 4]), dbuf("s5c_lo", [4, 128, T, 8, 4])]

    with tile.TileContext(nc) as tc, \
            tc.tile_pool(name="persist", bufs=1) as persist, \
            tc.tile_pool(name="ps", bufs=1, space="PSUM") as pspool:
        # ---- zero-fill pad buffers ----
        zt = persist.tile([128, 8192], f16)
        nc.vector.memset(zt[:], 0.0)
        for buf in ([a1, s2in] + s2b + s3in + s3b + s3c + s4in + s4b + s4c + s5in + s5b + s5c):
            sh = buf.shape
            if len(sh) == 4:
                per = sh[1] * sh[2] * sh[3]
                flat = buf.ap().rearrange("p a b c -> p (a b c)")
                for o in range(0, per, 8192):
                    n = min(8192, per - o)
                    nc.gpsimd.dma_start(flat[:, o:o + n], zt[:, :n])
            else:
                per = sh[2] * sh[3] * sh[4]
                for g in range(sh[0]):
                    flat = buf.ap()[g].rearrange("p a b c -> p (a b c)")
                    for o in range(0, per, 8192):
                        n = min(8192, per - o)
                        nc.gpsimd.dma_start(flat[:, o:o + n], zt[:, :n])

        # bias tiles
        bias_t = {}
        for (name, cin, cout, H, W, pool, mode) in CFGS:
            Go = max(1, cout // 128)
            bt = persist.tile([min(cout, 128), Go], f32, name=f"bt_{name}")
            nc.sync.dma_start(bt[:], bias_dr[name].ap())
            bias_t[name] = bt

        feats = persist.tile([128, 2, 4, T, 3], f16)

        # ================= stage 1 (per-frame path) =================
        with tc.tile_pool(name="s1w", bufs=1) as s1w:
            w11t = s1w.tile([108, 64], f16)
            nc.sync.dma_start(w11t[:], wdr["w_c11"].ap())
            w12t1 = s1w.tile([128, 9, 64], f16)
            w12t2 = s1w.tile([128, 9, 64], f16)
            nc.sync.dma_start(w12t1[:], wdr["w_c12_1"].ap())
            nc.sync.dma_start(w12t2[:], wdr["w_c12_2"].ap())

            with tc.tile_pool(name="s1a", bufs=1) as s1a:
                for f in range(T):
                    # ---- conv1_1 ----
                    imt = s1a.tile([108, 98, 34], f16, tag="imt", bufs=2, name=f"imt{f}")
                    nc.sync.dma_start(imt[:], i2c.ap()[:, f])
                    for ci, r0 in enumerate(range(0, 96, 16)):
                        p = pspool.tile([64, 512], f32, tag="mm", bufs=4, name=f"p11_{f}_{ci}")
                        nc.tensor.matmul(p[:].rearrange("c (a b) -> c a b", a=16),
                                         w11t[:], imt[:, r0:r0 + 16, 0:32],
                                         start=True, stop=True)
                        y = s1a.tile([64, 16, 32], f32, tag="y1", bufs=3, name=f"y11_{f}_{ci}")
                        nc.scalar.activation(y[:].rearrange("c a b -> c (a b)"), p[:],
                                             AF.Relu, bias=bias_t["c11"][:64, 0:1])
                        hi = s1a.tile([64, 16, 34], f16, tag="hi1", bufs=3, name=f"h11_{f}_{ci}")
                        lo = s1a.tile([64, 16, 34], f16, tag="lo1", bufs=3, name=f"l11_{f}_{ci}")
                        nc.vector.memset(hi[:, :, 0:1], 0.0)
                        nc.vector.memset(hi[:, :, 33:34], 0.0)
                        nc.vector.memset(lo[:, :, 0:1], 0.0)
                        nc.vector.memset(lo[:, :, 33:34], 0.0)
                        nc.vector.tensor_copy(hi[:, :, 1:33], y[:])
                        nc.vector.tensor_sub(lo[:, :, 1:33], y[:], hi[:, :, 1:33])
                        nc.sync.dma_start(a1.ap()[0:64, f, r0 + 1:r0 + 17, 0:34], hi[:])
                        nc.sync.dma_start(a1.ap()[64:128, f, r0 + 1:r0 + 17, 0:34], lo[:])
                    # ---- conv1_2 + pool1 ----
                    at = s1a.tile([128, 98, 34], f16, tag="at", bufs=2, name=f"at{f}")
                    nc.sync.dma_start(at[:], a1.ap()[:, f])
                    for ci, r0 in enumerate(range(0, 96, 16)):
                        p = pspool.tile([64, 512], f32, tag="mm", bufs=4, name=f"p12_{f}_{ci}")
                        mm = 0
                        for dy in range(3):
                            for dx in range(3):
                                t_ = dy * 3 + dx
                                rhs = at[:, r0 + dy:r0 + dy + 16, dx:dx + 32]
                                po = p[:].rearrange("c (a b) -> c a b", a=16)
                                nc.tensor.matmul(po, w12t1[:, t_, :], rhs,
                                                 start=(mm == 0), stop=False)
                                mm += 1
                                nc.tensor.matmul(po, w12t2[:, t_, :], rhs,
                                                 start=False, stop=(mm == 17))
                                mm += 1
                        y = s1a.tile([64, 16, 32], f32, tag="y1", bufs=3, name=f"y12_{f}_{ci}")
                        nc.scalar.activation(y[:].rearrange("c a b -> c (a b)"), p[:],
                                             AF.Relu, bias=bias_t["c12"][:64, 0:1])
                        yv = y[:].rearrange("c (h p) (w q) -> c h p w q", p=2, q=2)
                        m1 = s1a.tile([64, 8, 16], f32, tag="m1", bufs=3, name=f"m1_{f}_{ci}")
                        m2 = s1a.tile([64, 8, 16], f32, tag="m2", bufs=3, name=f"m2_{f}_{ci}")
                        nc.vector.tensor_tensor(m1[:], yv[:, :, 0, :, 0], yv[:, :, 0, :, 1], AO.max)
                        nc.vector.tensor_tensor(m2[:], yv[:, :, 1, :, 0], yv[:, :, 1, :, 1], AO.max)
                        nc.vector.tensor_tensor(m1[:], m1[:], m2[:], AO.max)
                        hi = s1a.tile([64, 8, 18], f16, tag="hi1p", bufs=3, name=f"h12_{f}_{ci}")
                        lo = s1a.tile([64, 8, 18], f16, tag="lo1p", bufs=3, name=f"l12_{f}_{ci}")
                        nc.vector.memset(hi[:, :, 0:1], 0.0)
                        nc.vector.memset(hi[:, :, 17:18], 0.0)
                        nc.vector.memset(lo[:, :, 0:1], 0.0)
                        nc.vector.memset(lo[:, :, 17:18], 0.0)
                        nc.vector.tensor_copy(hi[:, :, 1:17], m1[:])
                        nc.vector.tensor_sub(lo[:, :, 1:17], m1[:], hi[:, :, 1:17])
                        rp = r0 // 2
                        nc.sync.dma_start(s2in.ap()[0:64, f, rp + 1:rp + 9, 0:18], hi[:])
                        nc.sync.dma_start(s2in.ap()[64:128, f, rp + 1:rp + 9, 0:18], lo[:])

        # ================= stages 2-5 (whole-stage path) =================
        def conv_layer(name, cin, cout, H, W, pool, mode, src, dst):
            """src: stacked tensor (mode=='stk') or [hi, lo] grp'd tensors; dst likewise."""
            Hp, Wp = H + 2, W + 2
            Gi = max(1, cin // 128)
            Go = max(1, cout // 128)
            with tc.tile_pool(name=f"L_{name}", bufs=1) as lp:
                # load acts
                if mode == "stk":
                    at_hi = [lp.tile([128, T, Hp, Wp], f16, name=f"a_{name}")]
                    nc.sync.dma_start(at_hi[0][:], src.ap())
                    at_lo = None
                else:
                    at_hi, at_lo = [], []
                    for g in range(Gi):
                        th = lp.tile([128, T, Hp, Wp], f16, name=f"ah_{name}_{g}")
                        tl = lp.tile([128, T, Hp, Wp], f16, name=f"al_{name}_{g}")
                        nc.sync.dma_start(th[:], src[0].ap()[g])
                        nc.sync.dma_start(tl[:], src[1].ap()[g])
                        at_hi.append(th)
                        at_lo.append(tl)
                # load weights
                if mode == "stk":
                    w1 = lp.tile([128, 9, cout], f16, name=f"w1_{name}")
                    w2 = lp.tile([128, 9, cout], f16, name=f"w2_{name}")
                    nc.sync.dma_start(w1[:], wdr[f"w_{name}_1"].ap())
                    nc.sync.dma_start(w2[:], wdr[f"w_{name}_2"].ap())
                else:
                    whi_t = lp.tile([128, Gi, 9, cout], f16, name=f"wh_{name}")
                    wlo_t = lp.tile([128, Gi, 9, cout], f16, name=f"wl_{name}")
                    nc.sync.dma_start(whi_t[:], wdr[f"w_{name}_hi"].ap().rearrange("g p t c -> p g t c"))
                    nc.sync.dma_start(wlo_t[:], wdr[f"w_{name}_lo"].ap().rearrange("g p t c -> p g t c"))

                chunks = []
                if (H, W) == (48, 16):
                    for f in range(T):
                        for (kind, r0, rows) in CHUNKS[(48, 16)]:
                            chunks.append((f, 1, r0, rows))
                else:
                    for (kind, f0, fb) in CHUNKS[(H, W)]:
                        chunks.append((f0, fb, 0, H))

                for go in range(Go):
                    cosl = slice(go * 128, go * 128 + min(128, cout))
                    Mp = min(128, cout)
                    for (f0, fb, r0, rows) in chunks:
                        N = fb * rows * W
                        p = pspool.tile([128, 512], f32, tag="mm", bufs=4,
                                        name=f"p_{name}_{go}_{f0}_{r0}")
                        pv = p[:Mp, :N].rearrange("c (f a b) -> c f a b", f=fb, a=rows)
                        mm = 0
                        nmm = 9 * Gi * (2 if mode == "stk" else 3)
                        for gi in range(Gi):
                            for dy in range(3):
                                for dx in range(3):
                                    t_ = dy * 3 + dx
                                    if mode == "stk":
                                        rhs = at_hi[0][:, f0:f0 + fb, r0 + dy:r0 + dy + rows, dx:dx + W]
                                        nc.tensor.matmul(pv, w1[:, t_, cosl], rhs,
                                                         start=(mm == 0), stop=(mm == nmm - 1))
                                        mm += 1
                                        nc.tensor.matmul(pv, w2[:, t_, cosl], rhs,
                                                         start=False, stop=(mm == nmm - 1))
                                        mm += 1
                                    else:
                                        rh = at_hi[gi][:, f0:f0 + fb, r0 + dy:r0 + dy + rows, dx:dx + W]
                                        rl = at_lo[gi][:, f0:f0 + fb, r0 + dy:r0 + dy + rows, dx:dx + W]
                                        nc.tensor.matmul(pv, whi_t[:, gi, t_, cosl], rh,
                                                         start=(mm == 0), stop=False)
                                        mm += 1
                                        nc.tensor.matmul(pv, wlo_t[:, gi, t_, cosl], rh,
                                                         start=False, stop=False)
                                        mm += 1
                                        nc.tensor.matmul(pv, whi_t[:, gi, t_, cosl], rl,
                                                         start=False, stop=(mm == nmm - 1))
                                        mm += 1
                        y = lp.tile([128, fb, rows, W], f32, tag="y", bufs=3,
                                    name=f"y_{name}_{go}_{f0}_{r0}")
                        nc.scalar.activation(y[:Mp].rearrange("c f a b -> c (f a b)"), p[:Mp, :N],
                                             AF.Relu, bias=bias_t[name][:Mp, go:go + 1])
                        if not pool:
                            if rows == H:
                                # full-frame staging incl. pad rows/cols -> one contiguous DMA
                                hi = lp.tile([128, fb, H + 2, W + 2], f16, tag="sh", bufs=3, name=f"sh_{name}_{go}_{f0}_{r0}")
                                lo = lp.tile([128, fb, H + 2, W + 2], f16, tag="sl", bufs=3, name=f"sl_{name}_{go}_{f0}_{r0}")
                                for st in (hi, lo):
                                    nc.vector.memset(st[:Mp, :, 0:1, :], 0.0)
                                    nc.vector.memset(st[:Mp, :, H + 1:H + 2, :], 0.0)
                                    nc.vector.memset(st[:Mp, :, 1:H + 1, 0:1], 0.0)
                                    nc.vector.memset(st[:Mp, :, 1:H + 1, W + 1:W + 2], 0.0)
                                nc.vector.tensor_copy(hi[:Mp, :, 1:H + 1, 1:W + 1], y[:Mp])
                                nc.vector.tensor_sub(lo[:Mp, :, 1:H + 1, 1:W + 1], y[:Mp], hi[:Mp, :, 1:H + 1, 1:W + 1])
                                nc.sync.dma_start(dst[0].ap()[go, 0:Mp, f0:f0 + fb], hi[:Mp])
                                nc.sync.dma_start(dst[1].ap()[go, 0:Mp, f0:f0 + fb], lo[:Mp])
                            else:
                                # row-range staging, full width
                                hi = lp.tile([128, rows, W + 2], f16, tag="sh", bufs=3, name=f"sh_{name}_{go}_{f0}_{r0}")
                                lo = lp.tile([128, rows, W + 2], f16, tag="sl", bufs=3, name=f"sl_{name}_{go}_{f0}_{r0}")
                                for st in (hi, lo):
                                    nc.vector.memset(st[:Mp, :, 0:1], 0.0)
                                    nc.vector.memset(st[:Mp, :, W + 1:W + 2], 0.0)
                                nc.vector.tensor_copy(hi[:Mp, :, 1:W + 1], y[:Mp, 0])
                                nc.vector.tensor_sub(lo[:Mp, :, 1:W + 1], y[:Mp, 0], hi[:Mp, :, 1:W + 1])
                                nc.sync.dma_start(dst[0].ap()[go, 0:Mp, f0, r0 + 1:r0 + rows + 1, 0:W + 2], hi[:Mp])
                                nc.sync.dma_start(dst[1].ap()[go, 0:Mp, f0, r0 + 1:r0 + rows + 1, 0:W + 2], lo[:Mp])
                        else:
                            yv = y[:Mp].rearrange("c f (h p) (w q) -> c f h p w q", p=2, q=2)
                            ph, pw = rows // 2, W // 2
                            m1 = lp.tile([128, fb, ph, pw], f32, tag="pm1", bufs=3, name=f"pm1_{name}_{go}_{f0}_{r0}")
                            m2 = lp.tile([128, fb, ph, pw], f32, tag="pm2", bufs=3, name=f"pm2_{name}_{go}_{f0}_{r0}")
                            nc.vector.tensor_tensor(m1[:Mp], yv[:, :, :, 0, :, 0], yv[:, :, :, 0, :, 1], AO.max)
                            nc.vector.tensor_tensor(m2[:Mp], yv[:, :, :, 1, :, 0], yv[:, :, :, 1, :, 1], AO.max)
                            nc.vector.tensor_tensor(m1[:Mp], m1[:Mp], m2[:Mp], AO.max)
                            if name == "c53":
                                hi = lp.tile([128, fb, ph, pw], f16, tag="sh", bufs=3, name=f"sh_{name}_{go}_{f0}_{r0}")
                                lo = lp.tile([128, fb, ph, pw], f16, tag="sl", bufs=3, name=f"sl_{name}_{go}_{f0}_{r0}")
                                nc.vector.tensor_copy(hi[:Mp], m1[:Mp])
                                nc.vector.tensor_sub(lo[:Mp], m1[:Mp], hi[:Mp])
                                nc.vector.tensor_copy(feats[:, 0, go], hi[:Mp].rearrange("c f a b -> c f (a b)"))
                                nc.vector.tensor_copy(feats[:, 1, go], lo[:Mp].rearrange("c f a b -> c f (a b)"))
                            elif rows == H:
                                hi = lp.tile([128, fb, ph + 2, pw + 2], f16, tag="sh", bufs=3, name=f"sh_{name}_{go}_{f0}_{r0}")
                                lo = lp.tile([128, fb, ph + 2, pw + 2], f16, tag="sl", bufs=3, name=f"sl_{name}_{go}_{f0}_{r0}")
                                for st in (hi, lo):
                                    nc.vector.memset(st[:Mp, :, 0:1, :], 0.0)
                                    nc.vector.memset(st[:Mp, :, ph + 1:ph + 2, :], 0.0)
                                    nc.vector.memset(st[:Mp, :, 1:ph + 1, 0:1], 0.0)
                                    nc.vector.memset(st[:Mp, :, 1:ph + 1, pw + 1:pw + 2], 0.0)
                                nc.vector.tensor_copy(hi[:Mp, :, 1:ph + 1, 1:pw + 1], m1[:Mp])
                                nc.vector.tensor_sub(lo[:Mp, :, 1:ph + 1, 1:pw + 1], m1[:Mp], hi[:Mp, :, 1:ph + 1, 1:pw + 1])
                                nc.sync.dma_start(dst[0].ap()[go, 0:Mp, f0:f0 + fb], hi[:Mp])
                                nc.sync.dma_start(dst[1].ap()[go, 0:Mp, f0:f0 + fb], lo[:Mp])
                            else:
                                hi = lp.tile([128, ph, pw + 2], f16, tag="sh", bufs=3, name=f"sh_{name}_{go}_{f0}_{r0}")
                                lo = lp.tile([128, ph, pw + 2], f16, tag="sl", bufs=3, name=f"sl_{name}_{go}_{f0}_{r0}")
                                for st in (hi, lo):
                                    nc.vector.memset(st[:Mp, :, 0:1], 0.0)
                                    nc.vector.memset(st[:Mp, :, pw + 1:pw + 2], 0.0)
                                nc.vector.tensor_copy(hi[:Mp, :, 1:pw + 1], m1[:Mp, 0])
                                nc.vector.tensor_sub(lo[:Mp, :, 1:pw + 1], m1[:Mp, 0], hi[:Mp, :, 1:pw + 1])
                                rp = r0 // 2
                                nc.sync.dma_start(dst[0].ap()[go, 0:Mp, f0, rp + 1:rp + ph + 1, 0:pw + 2], hi[:Mp])
                                nc.sync.dma_start(dst[1].ap()[go, 0:Mp, f0, rp + 1:rp + ph + 1, 0:pw + 2], lo[:Mp])

        conv_layer("c21", 64, 128, 48, 16, False, "stk", s2in, s2b)
        conv_layer("c22", 128, 128, 48, 16, True, "3var", s2b, s3in)
        conv_layer("c31", 128, 256, 24, 8, False, "3var", s3in, s3b)
        conv_layer("c32", 256, 256, 24, 8, False, "3var", s3b, s3c)
        conv_layer("c33", 256, 256, 24, 8, True, "3var", s3c, s4in)
        conv_layer("c41", 256, 512, 12, 4, False, "3var", s4in, s4b)
        conv_layer("c42", 512, 512, 12, 4, False, "3var", s4b, s4c)
        conv_layer("c43", 512, 512, 12, 4, True, "3var", s4c, s5in)
        conv_layer("c51", 512, 512, 6, 2, False, "3var", s5in, s5b)
        conv_layer("c52", 512, 512, 6, 2, False, "3var", s5b, s5c)
        conv_layer("c53", 512, 512, 6, 2, True, "3var", s5c, None)

        # ================= LSTM + head =================
        import concourse.bass as bass
        with tc.tile_pool(name="lstm", bufs=1) as sb:
            wt = sb.tile([128, 3, 4, 2, 128], f16)
            nc.sync.dma_start(wt[:], wih.ap().rearrange("y g v c k -> c y g v k"))
            whht = sb.tile([HD, 4 * HD], f16)
            nc.sync.dma_start(whht[:], whh.ap())
            wlt = sb.tile([HD, 2, 2], f16)
            nc.sync.dma_start(wlt[:], wl.ap())

            pg = pspool.tile([T, 4 * HD], f32, tag="pg", name="pg")
            first = True
            for y in range(3):
                for g in range(4):
                    for (fv, wv) in ((0, 0), (0, 1), (1, 0)):
                        nc.tensor.matmul(pg[:], feats[:, fv, g, :, y], wt[:, y, g, wv, :],
                                         start=first, stop=(y == 2 and g == 3 and (fv, wv) == (1, 0)))
                        first = False
            binp32 = sb.tile([T, 4 * HD], f32)
            bsum_bc_t = sb.tile([T, 4 * HD], f32)
            nc.gpsimd.dma_start(out=bsum_bc_t[:], in_=bass.AP(
                tensor=bsum.ap().tensor, offset=0, ap=[[0, T], [1, 4 * HD]]))
            nc.vector.tensor_tensor(binp32[:], pg[:], bsum_bc_t[:], mybir.AluOpType.add)
            binp_flat = sb.tile([1, T * 4 * HD], f32)
            nc.sync.dma_start(binp_flat[:], binp32[:])

            h_sq = sb.tile([HD, HD], f16)
            h_sqT = sb.tile([HD, HD], f16)
            nc.vector.memset(h_sq[:], 0.0)
            nc.vector.memset(h_sqT[:], 0.0)
            cst = sb.tile([1, HD], f32)
            nc.vector.memset(cst[:], 0.0)
            gact = sb.tile([1, 4 * HD], f32)
            tmp1 = sb.tile([1, HD], f32)
            tmp2 = sb.tile([1, HD], f32)
            tanhc = sb.tile([1, HD], f32)
            h32 = sb.tile([1, HD], f32)
            HT = sb.tile([HD, T], f16)
            gsum = sb.tile([1, 4 * HD], f32)

            for t in range(T):
                pr = pspool.tile([1, 4 * HD], f32, tag="pr", bufs=2, name=f"pr{t}")
                nc.tensor.matmul(pr[:], h_sqT[:, 0:1], whht[:], start=True, stop=True)
                nc.vector.tensor_tensor(gsum[:], pr[:], binp_flat[:, t * 4 * HD:(t + 1) * 4 * HD],
                                        mybir.AluOpType.add)
                nc.scalar.activation(gact[:, 0:2 * HD], gsum[:, 0:2 * HD], AF.Sigmoid)
                nc.scalar.activation(gact[:, 2 * HD:3 * HD], gsum[:, 2 * HD:3 * HD], AF.Tanh)
                nc.scalar.activation(gact[:, 3 * HD:4 * HD], gsum[:, 3 * HD:4 * HD], AF.Sigmoid)
                nc.vector.tensor_tensor(tmp1[:], gact[:, HD:2 * HD], cst[:], mybir.AluOpType.mult)
                nc.vector.tensor_tensor(tmp2[:], gact[:, 0:HD], gact[:, 2 * HD:3 * HD], mybir.AluOpType.mult)
                nc.vector.tensor_tensor(cst[:], tmp1[:], tmp2[:], mybir.AluOpType.add)
                nc.scalar.activation(tanhc[:], cst[:], AF.Tanh)
                nc.vector.tensor_tensor(h32[:], gact[:, 3 * HD:4 * HD], tanhc[:], mybir.AluOpType.mult)
                nc.vector.tensor_copy(h_sq[0:1, :], h32[:])
                nc.vector.transpose(h_sqT[:], h_sq[:])
                nc.vector.tensor_copy(HT[:, t:t + 1], h_sqT[:, 0:1])

            HrT = sb.tile([HD, T], f16)
            nc.scalar.activation(HrT[:], HT[:], AF.Relu)
            Hr32 = sb.tile([HD, T], f32)
            nc.scalar.activation(Hr32[:], HT[:], AF.Relu)
            HrLo = sb.tile([HD, T], f16)
            nc.vector.tensor_sub(HrLo[:], Hr32[:], HrT[:])
            po = pspool.tile([T, 2], f32, tag="po", name="po")
            nc.tensor.matmul(po[:], HrT[:], wlt[:, 0, :], start=True, stop=False)
            nc.tensor.matmul(po[:], HrT[:], wlt[:, 1, :], start=False, stop=False)
            nc.tensor.matmul(po[:], HrLo[:], wlt[:, 0, :], start=False, stop=True)
            ot = sb.tile([T, 2], f32)
            bl_bc_t = sb.tile([T, 2], f32)
            nc.gpsimd.dma_start(out=bl_bc_t[:], in_=bass.AP(
                tensor=bl.ap().tensor, offset=0, ap=[[0, T], [1, 2]]))
            nc.vector.tensor_tensor(ot[:], po[:], bl_bc_t[:], mybir.AluOpType.add)
            nc.sync.dma_start(out.ap(), ot[:])

    nc.compile()
    return nc


def kernel(images, vgg_params, lstm_params, head_params):
    global _last_results
    import sys
    sys.path.insert(0, os.path.dirname(os.path.abspath(__file__)))
    try:
        import axon_prof
        axon_prof.install()
    except Exception:
        pass
    from concourse.bass_utils import run_bass_kernel_spmd

    images = np.asarray(images)
    B = images.shape[0]
    ins_common, per_core = _prep_host(images, vgg_params, lstm_params, head_params)
    nc = _build_module()
    in_maps = [{**ins_common, **per_core[b]} for b in range(B)]
    res = run_bass_kernel_spmd(nc, in_maps, core_ids=list(range(8)))
    _last_results = res
    standing = np.stack([res.results[b]["out"] for b in range(B)], axis=0).astype(np.float32)
    return (standing, standing, standing, standing)
